# revision 47
# baseline (speedup 1.0000x reference)
"""Trainium2 Bass kernel for nn_Decoder (worker/task label-probability decoder).

Math:
    worker_feature = inputs[:2048, :64]          # [Wn, A]
    tau            = inputs[2048:, :16]          # [T, L]
    p1 = sigmoid(x), x = worker_feature @ W + b  # [Wn, 1]
    p2 = (1 - p1) / (L - 1)
    P[i, j, l] = p1[i]^tau[j,l] * p2[i]^(1 - tau[j,l])
               = exp(a[i] * tau[j,l] + c[i])
      with  a = ln(p1/p2) = x + ln(L-1)   (exact: p1/(1-p1) = e^x)
            c = ln p2     = -(x + ln(1 + e^-x)) - ln(L-1)

Sharding: pure data parallel over the worker axis (dim 0), 256 workers per
core across 8 cores; tau/W/b replicated. No communication.

Per-core layout: workers on SBUF partitions (2 groups of 128), flattened
task axis (F = T*L = 32768) streamed in chunks. Each chunk of tau is
replicated to all 128 partitions by a GPSIMD indirect gather (all 128
row-indices equal) straight from HBM. The F axis is split into two
independent streams: most columns flow through the scalar engine
(Exp(a*tau + c) with per-partition scale/bias, one pass per worker group),
and a side stream flows through the vector engine using a bitcast-exp2
pipeline, so both engines compute exponentials concurrently. Outputs are
written as bf16 (the 2e-2 rel-err budget comfortably covers the ~2^-9
rounding plus the ~6e-3 bitcast-exp2 error) and upcast to fp32 on the
host. Output DMA traffic is spread over the SP and GPSIMD queues.
"""

import numpy as np

try:
    import concourse.bass as bass  # noqa: F401
except ImportError:  # fall back to the container's repo checkout
    import sys

    for _p in ("/root/.axon_site/_ro/trn_rl_repo", "/opt/trn_rl_repo"):
        if _p not in sys.path:
            sys.path.append(_p)

import concourse.bass as bass
import concourse.tile as tile
from concourse import library_config, mybir
from concourse.bass_utils import run_bass_kernel_spmd

WN = 2048  # workers total
TN = 2048  # tasks
L = 16  # edge types / labels
A = 64  # ability features
NCORES = 8
WPC = WN // NCORES  # workers per core (256)
G = WPC // 128  # partition groups per core (2)
F = TN * L  # flattened task axis (32768)

LN15 = float(np.log(np.float32(L - 1)).astype(np.float32))

# Schedule over the F axis: ("act"|"dve", size, n_out_sub_dmas). The two
# kinds form independent pipelines (scalar-engine exp vs vector-engine
# bitcast-exp2); interleaving them spreads the output DMAs in time. Small
# act chunks at the ends keep ramp and tail short. The dve chunks' tau
# replication goes through SP broadcast DMAs issued up front; act chunks
# use GPSIMD gathers.
import os

DVE_G = int(os.environ.get("DVE_G", "1408"))
_TAILA = F - 2048 - 3 * 8192 - 4 * DVE_G
SCHED = [
    ("act", 2048, 1),
    ("dve", DVE_G, 1),
    ("act", 8192, 2),
    ("dve", DVE_G, 1),
    ("act", 8192, 2),
    ("dve", DVE_G, 1),
    ("act", 8192, 8),
    ("dve", DVE_G, 1),
    ("act", _TAILA, 1),
]
assert sum(s[1] for s in SCHED) == F and _TAILA > 0
GATHER_STEP = 128  # row granularity of the tau gather (512B stride)
# 8192-chunks run as two half ACT ops (with their outs emitted per half) so
# replication, compute and writeback pipeline at half-chunk granularity.
ACT_SPLIT = {2, 6}

# Bitcast exp2: for t = z*log2(e) in (-127, 0], let y = int32(t*2^23 +
# 127*2^23). Bitcasting y to f32 gives s = 2^t * (1+f)/2^f where f is the
# fraction actually encoded in y's mantissa. Correct multiplicatively with
# g(f) = 2^f/(1+f), a degree-2 minimax polynomial of the mantissa integer
# m = y & 0x7fffff (exact in f32). Max rel err ~6.4e-3.
EXP_SCALE = float(np.log2(np.e) * (1 << 23))
EXP_BIAS = float(127.0 * (1 << 23))
Q2 = 0.22573194345762757 / (1 << 23) ** 2
Q1 = -0.2151853848831074 / (1 << 23)
Q0 = 0.993559438904892
MANT_MASK = 0x007FFFFF

_AF = mybir.ActivationFunctionType


class _TC(tile.TileContext):
    """TileContext legalized for a walrus that allows one sync-wait per inst.

    The walrus build in this container rejects any instruction carrying more
    than one sync-wait command. After Tile's normal scheduling + the exit
    drain/barrier, rewrite every multi-wait instruction into a chain of
    same-engine NOPs (one wait each) followed by the instruction with the
    final wait.
    """

    def _drain_and_barrier(self, tick_clock, wait_clock):
        super()._drain_and_barrier(tick_clock, wait_clock)
        self._split_multi_waits()

    def _fresh_nop(self, engine):
        inst = self.nc.engines[engine].nop(nofuse=True).ins
        self.nc.cur_bb.bb.instructions.remove(inst)
        return inst

    def _split_multi_waits(self):
        for fn in self.nc.m.functions:
            for bb in fn.blocks:
                snapshot = list(bb.instructions)
                if not any(
                    inst.sync_info and len(inst.sync_info.on_wait) > 1
                    for inst in snapshot
                ):
                    continue
                new = []
                for inst in snapshot:
                    si = inst.sync_info
                    if si is not None and si.on_wait and len(si.on_wait) > 1:
                        waits = list(si.on_wait)
                        si.on_wait = waits[-1:]
                        inst.sync_info = si
                        for wt in waits[:-1]:
                            nop = self._fresh_nop(inst.engine)
                            nop.sync_info = mybir.SyncInfo(on_wait=[wt], on_update=[])
                            new.append(nop)
                    new.append(inst)
                bb.instructions[:] = new


def build_nc():
    nc = bass.Bass("TRN2")
    wf = nc.dram_tensor("wf", [WPC, A], mybir.dt.float32, kind="ExternalInput")
    tau_in = nc.dram_tensor("tau", [F], mybir.dt.float32, kind="ExternalInput")
    # W (64) and b packed into one 512-byte row so a single broadcast-gather
    # fetches both with ~100ns latency.
    wb_in = nc.dram_tensor("wb", [128], mybir.dt.float32, kind="ExternalInput")
    out = nc.dram_tensor("out", [G, 128, F], mybir.dt.bfloat16, kind="ExternalOutput")

    f32 = mybir.dt.float32
    bf16 = mybir.dt.bfloat16
    i16 = mybir.dt.int16
    i32 = mybir.dt.int32

    # Output DMA issuers: SP carries ~5/8 of the write stream, GPSIMD
    # (which also runs the tau gathers) the rest.
    _OUT_PAT = "sgssgssg"

    def out_engine(i):
        return nc.sync if _OUT_PAT[i % len(_OUT_PAT)] == "s" else nc.gpsimd

    tau_ap = tau_in[:]
    n_uses = {}
    for kind, sz, _ in SCHED:
        n_uses[sz] = n_uses.get(sz, 0) + 1

    with _TC(nc) as tc:
        with (
            tc.tile_pool(name="const", bufs=1) as const,
            tc.tile_pool(name="main", bufs=2) as main,
        ):
            def gather_rep(k, f0, sz, nsub=1):
                """Replicate tau[f0:f0+sz] to all 128 partitions via dma_gather."""
                bufs = min(2, n_uses[sz])
                rep = main.tile(
                    [128, sz], f32, tag=f"rep{sz}", name=f"rep{k}", bufs=bufs
                )
                sub = sz // nsub
                assert f0 % GATHER_STEP == 0 and sub % GATHER_STEP == 0
                for j in range(nsub):
                    idx = const.tile([128, 8], i16, tag=f"idx{k}_{j}")
                    nc.vector.memset(idx, (f0 + j * sub) // GATHER_STEP)
                    nrows = (F - sub) // GATHER_STEP + 1
                    piece = rep[:, j * sub : (j + 1) * sub]
                    out3 = bass.AP(
                        tensor=piece.tensor,
                        offset=piece.offset,
                        ap=[list(piece.ap[0]), [sub, 1], [1, sub]],
                    )
                    in2 = bass.AP(
                        tensor=tau_ap.tensor,
                        offset=tau_ap.offset,
                        ap=[[GATHER_STEP, nrows], [1, sub]],
                    )
                    nc.gpsimd.dma_gather(
                        out_ap=out3,
                        in_ap=in2,
                        idxs_ap=idx[:],
                        num_idxs=128,
                        num_idxs_reg=128,
                        elem_size=sub,
                        elem_step=GATHER_STEP,
                    )
                return rep

            # ---- warm the ACT exp/ln table before anything depends on it ----
            dummy = const.tile([128, 1], f32)
            nc.vector.memset(dummy, 0.0)
            dummy2 = const.tile([128, 1], f32)
            nc.scalar.activation(dummy2, dummy, _AF.Exp)

            # GPSIMD ucode library with DMAGather, then the input gathers:
            # wf rows p+128g -> partition p (iota indices), W|b broadcast.
            # Gather latency is ~100ns vs ~1.7us for a dma_start, so the
            # whole preamble dependency chain starts almost immediately.
            idxgs = []
            for g in range(G):
                raw = const.tile([128, 8], i16, tag=f"idxwfr{g}")
                nc.gpsimd.iota(
                    raw, pattern=[[16, 8]], base=128 * g, channel_multiplier=1
                )
                # Only idx partitions 0-15 are consumed; mask the rest into
                # range (row count 256) to satisfy bounds checks.
                idxg = const.tile([128, 8], i16, tag=f"idxwf{g}")
                nc.vector.tensor_scalar(
                    idxg, raw, scalar1=WPC - 1, scalar2=None,
                    op0=mybir.AluOpType.bitwise_and,
                )
                idxgs.append(idxg)
            nc.gpsimd.load_library(library_config.mlp)

            wf_sb = const.tile([128, G, A], f32)
            wf_ap = wf[:]
            if os.environ.get("DBG_WF_DMA"):
                nc.sync.dma_start(
                    out=wf_sb, in_=wf[:].rearrange("(g p) a -> p g a", p=128)
                )
            else:
              for g in range(G):
                idxg = idxgs[g]
                piece = wf_sb[:, g, :]
                nc.gpsimd.dma_gather(
                    out_ap=bass.AP(
                        tensor=piece.tensor,
                        offset=piece.offset,
                        ap=[list(piece.ap[0]), [A, 1], [1, A]],
                    ),
                    in_ap=bass.AP(
                        tensor=wf_ap.tensor,
                        offset=wf_ap.offset,
                        ap=[[A, WPC], [1, A]],
                    ),
                    idxs_ap=idxg[:],
                    num_idxs=128,
                    num_idxs_reg=128,
                    elem_size=A,
                    elem_step=A,
                )
            wb_sb = const.tile([128, 128], f32)
            idx0 = const.tile([128, 8], i16, tag="idxwb")
            nc.vector.memset(idx0, 0)
            wb_ap = wb_in[:]
            if os.environ.get("DBG_WB_DMA"):
                nc.sync.dma_start(
                    out=wb_sb,
                    in_=bass.AP(tensor=wb_ap.tensor, offset=wb_ap.offset, ap=[[0, 128], [1, 128]]),
                )
            elif True:
              nc.gpsimd.dma_gather(
                out_ap=bass.AP(
                    tensor=wb_sb.tensor,
                    offset=wb_sb.offset,
                    ap=[list(wb_sb.ap[0]), [128, 1], [1, 128]],
                ),
                in_ap=bass.AP(
                    tensor=wb_ap.tensor, offset=wb_ap.offset, ap=[[128, 1], [1, 128]]
                ),
                idxs_ap=idx0[:],
                num_idxs=128,
                num_idxs_reg=128,
                elem_size=128,
                elem_step=128,
            )
            w_sb = wb_sb[:, :A]
            b_sb = wb_sb[:, A : A + 1]

            rep_first = gather_rep(0, 0, SCHED[0][1])

            dve_off = {}
            f0 = 0
            for k, (kind, sz, _) in enumerate(SCHED):
                dve_off[k] = f0
                f0 += sz

            # first dve chunk's tau replication on the otherwise-idle SP
            # queue at t=0; the later dve chunks use gathers like act chunks.
            rep_d1 = main.tile(
                [128, SCHED[1][1]], f32, tag=f"rep{SCHED[1][1]}", name="rep1", bufs=2
            )
            nc.sync.dma_start(
                out=rep_d1,
                in_=bass.AP(
                    tensor=tau_ap.tensor,
                    offset=tau_ap.offset + dve_off[1],
                    ap=[[0, 128], [1, SCHED[1][1]]],
                ),
            )
            dve_reps = {1: rep_d1}

            # ---- per-worker scalars: a = x + ln15, c = -(x + b + ln(1+e^-(x+b))) - ln15
            x = const.tile([128, G], f32)
            for g in range(G):
                prod = const.tile([128, A], f32, tag=f"prod{g}")
                nc.vector.tensor_mul(prod, wf_sb[:, g, :], w_sb)
                nc.vector.reduce_sum(x[:, g : g + 1], prod, axis=mybir.AxisListType.X)

            xb = const.tile([128, G], f32)
            nc.vector.tensor_scalar(
                xb, x, scalar1=b_sb, scalar2=None, op0=mybir.AluOpType.add
            )
            e = const.tile([128, G], f32)
            nc.scalar.activation(e, xb, _AF.Exp, scale=-1.0)
            s = const.tile([128, G], f32)
            nc.vector.tensor_scalar_add(s, e, 1.0)
            ls = const.tile([128, G], f32)
            nc.scalar.activation(ls, s, _AF.Ln)
            u = const.tile([128, G], f32)
            nc.vector.tensor_add(u, xb, ls)
            c_sb = const.tile([128, G], f32)
            nc.vector.tensor_scalar(
                c_sb,
                u,
                scalar1=-1.0,
                scalar2=-LN15,
                op0=mybir.AluOpType.mult,
                op1=mybir.AluOpType.add,
            )
            a_sb = const.tile([128, G], f32)
            nc.vector.tensor_scalar_add(a_sb, xb, LN15)

            # scaled affine constants for the DVE bitcast-exp2 path
            a_sc = const.tile([128, G], f32)
            nc.vector.tensor_scalar_mul(a_sc, a_sb, EXP_SCALE)
            ccorr = const.tile([128, G], f32)
            nc.vector.tensor_scalar(
                ccorr,
                c_sb,
                scalar1=EXP_SCALE,
                scalar2=EXP_BIAS,
                op0=mybir.AluOpType.mult,
                op1=mybir.AluOpType.add,
            )

            def dve_exp(ot, rep, d, g, key):
                # bf16 poly intermediates hit the DVE 2x (2-byte packed)
                # mode; precision stays within budget (max ~1.1e-2 overall).
                y = main.tile([128, d], i32, tag=f"dve_y{d}", name=f"y{key}", bufs=1)
                nc.vector.tensor_scalar(
                    y,
                    rep,
                    scalar1=a_sc[:, g : g + 1],
                    scalar2=ccorr[:, g : g + 1],
                    op0=mybir.AluOpType.mult,
                    op1=mybir.AluOpType.add,
                )
                m = main.tile([128, d], i32, tag=f"dve_m{d}", name=f"m{key}", bufs=1)
                nc.vector.tensor_scalar(
                    m, y, scalar1=MANT_MASK, scalar2=None, op0=mybir.AluOpType.bitwise_and
                )
                fm = main.tile([128, d], bf16, tag=f"dve_fm{d}", name=f"fm{key}", bufs=1)
                nc.vector.tensor_scalar_add(fm, m, 0.0)
                h1 = main.tile([128, d], bf16, tag=f"dve_h1{d}", name=f"h1{key}", bufs=1)
                nc.vector.tensor_scalar(
                    h1,
                    fm,
                    scalar1=Q2,
                    scalar2=Q1,
                    op0=mybir.AluOpType.mult,
                    op1=mybir.AluOpType.add,
                )
                h2 = main.tile([128, d], bf16, tag=f"dve_h2{d}", name=f"h2{key}", bufs=1)
                nc.vector.tensor_mul(h2, h1, fm)
                h3 = main.tile([128, d], bf16, tag=f"dve_h3{d}", name=f"h3{key}", bufs=1)
                nc.vector.tensor_scalar_add(h3, h2, Q0)
                nc.vector.tensor_mul(ot, h3, y.bitcast(f32))

            # ---- main loop ----
            out_i = 0
            f0 = 0
            for k, (kind, sz, nout) in enumerate(SCHED):
                if k == 0:
                    rep = rep_first
                elif k in dve_reps:
                    rep = dve_reps[k]
                else:
                    rep = gather_rep(k, f0, sz, nsub=2 if k in ACT_SPLIT else 1)
                bufs = min(2, n_uses[sz])
                for g in range(G):
                    ot = main.tile(
                        [128, sz],
                        bf16,
                        tag=f"ot{sz}g{g}",
                        name=f"ot{k}g{g}",
                        bufs=bufs,
                    )
                    osub = sz // nout

                    def emit_outs(lo, hi, g=g, ot=ot, base=f0):
                        nonlocal out_i
                        for j in range(lo, hi):
                            out_engine(out_i).dma_start(
                                out=out[g, :, base + j * osub : base + (j + 1) * osub],
                                in_=ot[:, j * osub : (j + 1) * osub],
                            )
                            out_i += 1

                    if kind == "act":
                        nsp = 2 if k in ACT_SPLIT else 1
                        half = sz // nsp
                        for h in range(nsp):
                            nc.scalar.activation(
                                ot[:, h * half : (h + 1) * half],
                                rep[:, h * half : (h + 1) * half],
                                _AF.Exp,
                                bias=c_sb[:, g : g + 1],
                                scale=a_sb[:, g : g + 1],
                            )
                            emit_outs(
                                h * nout // nsp, (h + 1) * nout // nsp
                            )
                    else:
                        dve_exp(ot, rep, sz, g, f"{k}g{g}")
                        emit_outs(0, nout)
                f0 += sz
    return nc


_NC = None


def kernel(inputs, W, b, worker_num=WN, task_num=TN, edge_type=L, ability_num=A, **_kw):
    global _NC
    inputs = np.ascontiguousarray(np.asarray(inputs, dtype=np.float32))
    W = np.asarray(W, dtype=np.float32).reshape(A)
    b = np.asarray(b, dtype=np.float32).reshape(1)
    assert inputs.shape == (WN + TN, A)

    wf = inputs[:WN, :A]
    tau = np.ascontiguousarray(inputs[WN:, :L].reshape(F))
    wb = np.zeros(128, dtype=np.float32)
    wb[:A] = W
    wb[A] = b[0]

    if _NC is None:
        _NC = build_nc()

    in_maps = [
        {
            "wf": np.ascontiguousarray(wf[k * WPC : (k + 1) * WPC]),
            "tau": tau,
            "wb": wb,
        }
        for k in range(NCORES)
    ]
    res = run_bass_kernel_spmd(_NC, in_maps, core_ids=list(range(NCORES)))
    parts = [
        np.asarray(r["out"]).astype(np.float32).reshape(WPC, TN, L)
        for r in res.results
    ]
    return np.concatenate(parts, axis=0)


# revision 48
# speedup vs baseline: 1.0000x; 1.0000x over previous
"""Trainium2 Bass kernel for nn_Decoder (worker/task label-probability decoder).

Math:
    worker_feature = inputs[:2048, :64]          # [Wn, A]
    tau            = inputs[2048:, :16]          # [T, L]
    p1 = sigmoid(x), x = worker_feature @ W + b  # [Wn, 1]
    p2 = (1 - p1) / (L - 1)
    P[i, j, l] = p1[i]^tau[j,l] * p2[i]^(1 - tau[j,l])
               = exp(a[i] * tau[j,l] + c[i])
      with  a = ln(p1/p2) = x + ln(L-1)   (exact: p1/(1-p1) = e^x)
            c = ln p2     = -(x + ln(1 + e^-x)) - ln(L-1)

Sharding: pure data parallel over the worker axis (dim 0), 256 workers per
core across 8 cores; tau/W/b replicated. No communication.

Per-core layout: workers on SBUF partitions (2 groups of 128), flattened
task axis (F = T*L = 32768) streamed in chunks. Each chunk of tau is
replicated to all 128 partitions by a GPSIMD indirect gather (all 128
row-indices equal) straight from HBM. The F axis is split into two
independent streams: most columns flow through the scalar engine
(Exp(a*tau + c) with per-partition scale/bias, one pass per worker group),
and a side stream flows through the vector engine using a bitcast-exp2
pipeline, so both engines compute exponentials concurrently. Outputs are
written as bf16 (the 2e-2 rel-err budget comfortably covers the ~2^-9
rounding plus the ~6e-3 bitcast-exp2 error) and upcast to fp32 on the
host. Output DMA traffic is spread over the SP and GPSIMD queues.
"""

import numpy as np

try:
    import concourse.bass as bass  # noqa: F401
except ImportError:  # fall back to the container's repo checkout
    import sys

    for _p in ("/root/.axon_site/_ro/trn_rl_repo", "/opt/trn_rl_repo"):
        if _p not in sys.path:
            sys.path.append(_p)

import concourse.bass as bass
import concourse.tile as tile
from concourse import library_config, mybir
from concourse.bass_utils import run_bass_kernel_spmd

WN = 2048  # workers total
TN = 2048  # tasks
L = 16  # edge types / labels
A = 64  # ability features
NCORES = 8
WPC = WN // NCORES  # workers per core (256)
G = WPC // 128  # partition groups per core (2)
F = TN * L  # flattened task axis (32768)

LN15 = float(np.log(np.float32(L - 1)).astype(np.float32))

# Schedule over the F axis: ("act"|"dve", size, n_out_sub_dmas). The two
# kinds form independent pipelines (scalar-engine exp vs vector-engine
# bitcast-exp2); interleaving them spreads the output DMAs in time. Small
# act chunks at the ends keep ramp and tail short. The dve chunks' tau
# replication goes through SP broadcast DMAs issued up front; act chunks
# use GPSIMD gathers.
import os

DVE_G = int(os.environ.get("DVE_G", "1408"))
_TAILA = F - 2048 - 3 * 8192 - 4 * DVE_G
SCHED = [
    ("act", 2048, 1),
    ("dve", DVE_G, 1),
    ("act", 8192, 2),
    ("dve", DVE_G, 1),
    ("act", 8192, 2),
    ("dve", DVE_G, 1),
    ("act", 8192, 4),
    ("dve", DVE_G, 1),
    ("act", _TAILA, 1),
]
assert sum(s[1] for s in SCHED) == F and _TAILA > 0
GATHER_STEP = 128  # row granularity of the tau gather (512B stride)
# 8192-chunks run as two half ACT ops (with their outs emitted per half) so
# replication, compute and writeback pipeline at half-chunk granularity.
ACT_SPLIT = {2, 6}

# Bitcast exp2: for t = z*log2(e) in (-127, 0], let y = int32(t*2^23 +
# 127*2^23). Bitcasting y to f32 gives s = 2^t * (1+f)/2^f where f is the
# fraction actually encoded in y's mantissa. Correct multiplicatively with
# g(f) = 2^f/(1+f), a degree-2 minimax polynomial of the mantissa integer
# m = y & 0x7fffff (exact in f32). Max rel err ~6.4e-3.
EXP_SCALE = float(np.log2(np.e) * (1 << 23))
EXP_BIAS = float(127.0 * (1 << 23))
Q2 = 0.22573194345762757 / (1 << 23) ** 2
Q1 = -0.2151853848831074 / (1 << 23)
Q0 = 0.993559438904892
MANT_MASK = 0x007FFFFF

_AF = mybir.ActivationFunctionType


class _TC(tile.TileContext):
    """TileContext legalized for a walrus that allows one sync-wait per inst.

    The walrus build in this container rejects any instruction carrying more
    than one sync-wait command. After Tile's normal scheduling + the exit
    drain/barrier, rewrite every multi-wait instruction into a chain of
    same-engine NOPs (one wait each) followed by the instruction with the
    final wait.
    """

    def _drain_and_barrier(self, tick_clock, wait_clock):
        super()._drain_and_barrier(tick_clock, wait_clock)
        self._split_multi_waits()

    def _fresh_nop(self, engine):
        inst = self.nc.engines[engine].nop(nofuse=True).ins
        self.nc.cur_bb.bb.instructions.remove(inst)
        return inst

    def _split_multi_waits(self):
        for fn in self.nc.m.functions:
            for bb in fn.blocks:
                snapshot = list(bb.instructions)
                if not any(
                    inst.sync_info and len(inst.sync_info.on_wait) > 1
                    for inst in snapshot
                ):
                    continue
                new = []
                for inst in snapshot:
                    si = inst.sync_info
                    if si is not None and si.on_wait and len(si.on_wait) > 1:
                        waits = list(si.on_wait)
                        si.on_wait = waits[-1:]
                        inst.sync_info = si
                        for wt in waits[:-1]:
                            nop = self._fresh_nop(inst.engine)
                            nop.sync_info = mybir.SyncInfo(on_wait=[wt], on_update=[])
                            new.append(nop)
                    new.append(inst)
                bb.instructions[:] = new


def build_nc():
    nc = bass.Bass("TRN2")
    wf = nc.dram_tensor("wf", [WPC, A], mybir.dt.float32, kind="ExternalInput")
    tau_in = nc.dram_tensor("tau", [F], mybir.dt.float32, kind="ExternalInput")
    # W (64) and b packed into one 512-byte row so a single broadcast-gather
    # fetches both with ~100ns latency.
    wb_in = nc.dram_tensor("wb", [128], mybir.dt.float32, kind="ExternalInput")
    out = nc.dram_tensor("out", [G, 128, F], mybir.dt.bfloat16, kind="ExternalOutput")

    f32 = mybir.dt.float32
    bf16 = mybir.dt.bfloat16
    i16 = mybir.dt.int16
    i32 = mybir.dt.int32

    # Output DMA issuers: SP carries ~5/8 of the write stream, GPSIMD
    # (which also runs the tau gathers) the rest.
    _OUT_PAT = "sgssgssg"

    def out_engine(i):
        return nc.sync if _OUT_PAT[i % len(_OUT_PAT)] == "s" else nc.gpsimd

    tau_ap = tau_in[:]
    n_uses = {}
    for kind, sz, _ in SCHED:
        n_uses[sz] = n_uses.get(sz, 0) + 1

    with _TC(nc) as tc:
        with (
            tc.tile_pool(name="const", bufs=1) as const,
            tc.tile_pool(name="main", bufs=2) as main,
        ):
            def gather_rep(k, f0, sz, nsub=1):
                """Replicate tau[f0:f0+sz] to all 128 partitions via dma_gather."""
                bufs = min(2, n_uses[sz])
                rep = main.tile(
                    [128, sz], f32, tag=f"rep{sz}", name=f"rep{k}", bufs=bufs
                )
                sub = sz // nsub
                assert f0 % GATHER_STEP == 0 and sub % GATHER_STEP == 0
                for j in range(nsub):
                    idx = const.tile([128, 8], i16, tag=f"idx{k}_{j}")
                    nc.vector.memset(idx, (f0 + j * sub) // GATHER_STEP)
                    nrows = (F - sub) // GATHER_STEP + 1
                    piece = rep[:, j * sub : (j + 1) * sub]
                    out3 = bass.AP(
                        tensor=piece.tensor,
                        offset=piece.offset,
                        ap=[list(piece.ap[0]), [sub, 1], [1, sub]],
                    )
                    in2 = bass.AP(
                        tensor=tau_ap.tensor,
                        offset=tau_ap.offset,
                        ap=[[GATHER_STEP, nrows], [1, sub]],
                    )
                    nc.gpsimd.dma_gather(
                        out_ap=out3,
                        in_ap=in2,
                        idxs_ap=idx[:],
                        num_idxs=128,
                        num_idxs_reg=128,
                        elem_size=sub,
                        elem_step=GATHER_STEP,
                    )
                return rep

            # ---- warm the ACT exp/ln table before anything depends on it ----
            dummy = const.tile([128, 1], f32)
            nc.vector.memset(dummy, 0.0)
            dummy2 = const.tile([128, 1], f32)
            nc.scalar.activation(dummy2, dummy, _AF.Exp)

            # GPSIMD ucode library with DMAGather, then the input gathers:
            # wf rows p+128g -> partition p (iota indices), W|b broadcast.
            # Gather latency is ~100ns vs ~1.7us for a dma_start, so the
            # whole preamble dependency chain starts almost immediately.
            idxgs = []
            for g in range(G):
                raw = const.tile([128, 8], i16, tag=f"idxwfr{g}")
                nc.gpsimd.iota(
                    raw, pattern=[[16, 8]], base=128 * g, channel_multiplier=1
                )
                # Only idx partitions 0-15 are consumed; mask the rest into
                # range (row count 256) to satisfy bounds checks.
                idxg = const.tile([128, 8], i16, tag=f"idxwf{g}")
                nc.vector.tensor_scalar(
                    idxg, raw, scalar1=WPC - 1, scalar2=None,
                    op0=mybir.AluOpType.bitwise_and,
                )
                idxgs.append(idxg)
            nc.gpsimd.load_library(library_config.mlp)

            wf_sb = const.tile([128, G, A], f32)
            wf_ap = wf[:]
            for g in range(G):
                idxg = idxgs[g]
                piece = wf_sb[:, g, :]
                nc.gpsimd.dma_gather(
                    out_ap=bass.AP(
                        tensor=piece.tensor,
                        offset=piece.offset,
                        ap=[list(piece.ap[0]), [A, 1], [1, A]],
                    ),
                    in_ap=bass.AP(
                        tensor=wf_ap.tensor,
                        offset=wf_ap.offset,
                        ap=[[A, WPC], [1, A]],
                    ),
                    idxs_ap=idxg[:],
                    num_idxs=128,
                    num_idxs_reg=128,
                    elem_size=A,
                    elem_step=A,
                )
            wb_sb = const.tile([128, 128], f32)
            idx0 = const.tile([128, 8], i16, tag="idxwb")
            nc.vector.memset(idx0, 0)
            wb_ap = wb_in[:]
            nc.gpsimd.dma_gather(
                out_ap=bass.AP(
                    tensor=wb_sb.tensor,
                    offset=wb_sb.offset,
                    ap=[list(wb_sb.ap[0]), [128, 1], [1, 128]],
                ),
                in_ap=bass.AP(
                    tensor=wb_ap.tensor, offset=wb_ap.offset, ap=[[128, 1], [1, 128]]
                ),
                idxs_ap=idx0[:],
                num_idxs=128,
                num_idxs_reg=128,
                elem_size=128,
                elem_step=128,
            )
            w_sb = wb_sb[:, :A]
            b_sb = wb_sb[:, A : A + 1]

            rep_first = gather_rep(0, 0, SCHED[0][1])

            dve_off = {}
            f0 = 0
            for k, (kind, sz, _) in enumerate(SCHED):
                dve_off[k] = f0
                f0 += sz

            # first dve chunk's tau replication on the otherwise-idle SP
            # queue at t=0; the later dve chunks use gathers like act chunks.
            rep_d1 = main.tile(
                [128, SCHED[1][1]], f32, tag=f"rep{SCHED[1][1]}", name="rep1", bufs=2
            )
            nc.sync.dma_start(
                out=rep_d1,
                in_=bass.AP(
                    tensor=tau_ap.tensor,
                    offset=tau_ap.offset + dve_off[1],
                    ap=[[0, 128], [1, SCHED[1][1]]],
                ),
            )
            dve_reps = {1: rep_d1}

            # ---- per-worker scalars: a = x + ln15, c = -(x + b + ln(1+e^-(x+b))) - ln15
            x = const.tile([128, G], f32)
            for g in range(G):
                prod = const.tile([128, A], f32, tag=f"prod{g}")
                nc.vector.tensor_mul(prod, wf_sb[:, g, :], w_sb)
                nc.vector.reduce_sum(x[:, g : g + 1], prod, axis=mybir.AxisListType.X)

            xb = const.tile([128, G], f32)
            nc.vector.tensor_scalar(
                xb, x, scalar1=b_sb, scalar2=None, op0=mybir.AluOpType.add
            )
            e = const.tile([128, G], f32)
            nc.scalar.activation(e, xb, _AF.Exp, scale=-1.0)
            s = const.tile([128, G], f32)
            nc.vector.tensor_scalar_add(s, e, 1.0)
            ls = const.tile([128, G], f32)
            nc.scalar.activation(ls, s, _AF.Ln)
            u = const.tile([128, G], f32)
            nc.vector.tensor_add(u, xb, ls)
            c_sb = const.tile([128, G], f32)
            nc.vector.tensor_scalar(
                c_sb,
                u,
                scalar1=-1.0,
                scalar2=-LN15,
                op0=mybir.AluOpType.mult,
                op1=mybir.AluOpType.add,
            )
            a_sb = const.tile([128, G], f32)
            nc.vector.tensor_scalar_add(a_sb, xb, LN15)

            # scaled affine constants for the DVE bitcast-exp2 path
            a_sc = const.tile([128, G], f32)
            nc.vector.tensor_scalar_mul(a_sc, a_sb, EXP_SCALE)
            ccorr = const.tile([128, G], f32)
            nc.vector.tensor_scalar(
                ccorr,
                c_sb,
                scalar1=EXP_SCALE,
                scalar2=EXP_BIAS,
                op0=mybir.AluOpType.mult,
                op1=mybir.AluOpType.add,
            )

            def dve_exp(ot, rep, d, g, key):
                # bf16 poly intermediates hit the DVE 2x (2-byte packed)
                # mode; precision stays within budget (max ~1.1e-2 overall).
                y = main.tile([128, d], i32, tag=f"dve_y{d}", name=f"y{key}", bufs=1)
                nc.vector.tensor_scalar(
                    y,
                    rep,
                    scalar1=a_sc[:, g : g + 1],
                    scalar2=ccorr[:, g : g + 1],
                    op0=mybir.AluOpType.mult,
                    op1=mybir.AluOpType.add,
                )
                m = main.tile([128, d], i32, tag=f"dve_m{d}", name=f"m{key}", bufs=1)
                nc.vector.tensor_scalar(
                    m, y, scalar1=MANT_MASK, scalar2=None, op0=mybir.AluOpType.bitwise_and
                )
                fm = main.tile([128, d], bf16, tag=f"dve_fm{d}", name=f"fm{key}", bufs=1)
                nc.vector.tensor_scalar_add(fm, m, 0.0)
                h1 = main.tile([128, d], bf16, tag=f"dve_h1{d}", name=f"h1{key}", bufs=1)
                nc.vector.tensor_scalar(
                    h1,
                    fm,
                    scalar1=Q2,
                    scalar2=Q1,
                    op0=mybir.AluOpType.mult,
                    op1=mybir.AluOpType.add,
                )
                h2 = main.tile([128, d], bf16, tag=f"dve_h2{d}", name=f"h2{key}", bufs=1)
                nc.vector.tensor_mul(h2, h1, fm)
                h3 = main.tile([128, d], bf16, tag=f"dve_h3{d}", name=f"h3{key}", bufs=1)
                nc.vector.tensor_scalar_add(h3, h2, Q0)
                nc.vector.tensor_mul(ot, h3, y.bitcast(f32))

            # ---- main loop ----
            out_i = 0
            f0 = 0
            for k, (kind, sz, nout) in enumerate(SCHED):
                if k == 0:
                    rep = rep_first
                elif k in dve_reps:
                    rep = dve_reps[k]
                else:
                    rep = gather_rep(k, f0, sz, nsub=2 if k in ACT_SPLIT else 1)
                bufs = min(2, n_uses[sz])
                for g in range(G):
                    ot = main.tile(
                        [128, sz],
                        bf16,
                        tag=f"ot{sz}g{g}",
                        name=f"ot{k}g{g}",
                        bufs=bufs,
                    )
                    osub = sz // nout

                    def emit_outs(lo, hi, g=g, ot=ot, base=f0):
                        nonlocal out_i
                        for j in range(lo, hi):
                            out_engine(out_i).dma_start(
                                out=out[g, :, base + j * osub : base + (j + 1) * osub],
                                in_=ot[:, j * osub : (j + 1) * osub],
                            )
                            out_i += 1

                    if kind == "act":
                        nsp = 2 if k in ACT_SPLIT else 1
                        half = sz // nsp
                        for h in range(nsp):
                            nc.scalar.activation(
                                ot[:, h * half : (h + 1) * half],
                                rep[:, h * half : (h + 1) * half],
                                _AF.Exp,
                                bias=c_sb[:, g : g + 1],
                                scale=a_sb[:, g : g + 1],
                            )
                            emit_outs(
                                h * nout // nsp, (h + 1) * nout // nsp
                            )
                    else:
                        dve_exp(ot, rep, sz, g, f"{k}g{g}")
                        emit_outs(0, nout)
                f0 += sz
    return nc


_NC = None


def kernel(inputs, W, b, worker_num=WN, task_num=TN, edge_type=L, ability_num=A, **_kw):
    global _NC
    inputs = np.ascontiguousarray(np.asarray(inputs, dtype=np.float32))
    W = np.asarray(W, dtype=np.float32).reshape(A)
    b = np.asarray(b, dtype=np.float32).reshape(1)
    assert inputs.shape == (WN + TN, A)

    wf = inputs[:WN, :A]
    tau = np.ascontiguousarray(inputs[WN:, :L].reshape(F))
    wb = np.zeros(128, dtype=np.float32)
    wb[:A] = W
    wb[A] = b[0]

    if _NC is None:
        _NC = build_nc()

    in_maps = [
        {
            "wf": np.ascontiguousarray(wf[k * WPC : (k + 1) * WPC]),
            "tau": tau,
            "wb": wb,
        }
        for k in range(NCORES)
    ]
    res = run_bass_kernel_spmd(_NC, in_maps, core_ids=list(range(NCORES)))
    parts = [
        np.asarray(r["out"]).astype(np.float32).reshape(WPC, TN, L)
        for r in res.results
    ]
    return np.concatenate(parts, axis=0)


# revision 49
# speedup vs baseline: 1.0013x; 1.0013x over previous
"""Trainium2 Bass kernel for nn_Decoder (worker/task label-probability decoder).

Math:
    worker_feature = inputs[:2048, :64]          # [Wn, A]
    tau            = inputs[2048:, :16]          # [T, L]
    p1 = sigmoid(x), x = worker_feature @ W + b  # [Wn, 1]
    p2 = (1 - p1) / (L - 1)
    P[i, j, l] = p1[i]^tau[j,l] * p2[i]^(1 - tau[j,l])
               = exp(a[i] * tau[j,l] + c[i])
      with  a = ln(p1/p2) = x + ln(L-1)   (exact: p1/(1-p1) = e^x)
            c = ln p2     = -(x + ln(1 + e^-x)) - ln(L-1)

Sharding: pure data parallel over the worker axis (dim 0), 256 workers per
core across 8 cores; tau/W/b replicated. No communication.

Per-core layout: workers on SBUF partitions (2 groups of 128), flattened
task axis (F = T*L = 32768) streamed in chunks. Each chunk of tau is
replicated to all 128 partitions by a GPSIMD indirect gather (all 128
row-indices equal) straight from HBM. The F axis is split into two
independent streams: most columns flow through the scalar engine
(Exp(a*tau + c) with per-partition scale/bias, one pass per worker group),
and a side stream flows through the vector engine using a bitcast-exp2
pipeline, so both engines compute exponentials concurrently. Outputs are
written as bf16 (the 2e-2 rel-err budget comfortably covers the ~2^-9
rounding plus the ~6e-3 bitcast-exp2 error) and upcast to fp32 on the
host. Output DMA traffic is spread over the SP and GPSIMD queues.
"""

import numpy as np

try:
    import concourse.bass as bass  # noqa: F401
except ImportError:  # fall back to the container's repo checkout
    import sys

    for _p in ("/root/.axon_site/_ro/trn_rl_repo", "/opt/trn_rl_repo"):
        if _p not in sys.path:
            sys.path.append(_p)

import concourse.bass as bass
import concourse.tile as tile
from concourse import library_config, mybir
from concourse.bass_utils import run_bass_kernel_spmd

WN = 2048  # workers total
TN = 2048  # tasks
L = 16  # edge types / labels
A = 64  # ability features
NCORES = 8
WPC = WN // NCORES  # workers per core (256)
G = WPC // 128  # partition groups per core (2)
F = TN * L  # flattened task axis (32768)

LN15 = float(np.log(np.float32(L - 1)).astype(np.float32))

# Schedule over the F axis: ("act"|"dve", size, n_out_sub_dmas). The two
# kinds form independent pipelines (scalar-engine exp vs vector-engine
# bitcast-exp2); interleaving them spreads the output DMAs in time. Small
# act chunks at the ends keep ramp and tail short. The dve chunks' tau
# replication goes through SP broadcast DMAs issued up front; act chunks
# use GPSIMD gathers.
import os

DVE_G = int(os.environ.get("DVE_G", "1408"))
_TAILA = F - 2048 - 3 * 8192 - 4 * DVE_G
SCHED = [
    ("act", 2048, 1),
    ("dve", DVE_G, 1),
    ("act", 8192, 2),
    ("dve", DVE_G, 1),
    ("act", 8192, 2),
    ("dve", DVE_G, 1),
    ("act", 8192, 4),
    ("dve", DVE_G, 1),
    ("act", _TAILA, 1),
]
assert sum(s[1] for s in SCHED) == F and _TAILA > 0
GATHER_STEP = 128  # row granularity of the tau gather (512B stride)
# 8192-chunks run as two half ACT ops (with their outs emitted per half) so
# replication, compute and writeback pipeline at half-chunk granularity.
ACT_SPLIT = {2, 6}

# Bitcast exp2: for t = z*log2(e) in (-127, 0], let y = int32(t*2^23 +
# 127*2^23). Bitcasting y to f32 gives s = 2^t * (1+f)/2^f where f is the
# fraction actually encoded in y's mantissa. Correct multiplicatively with
# g(f) = 2^f/(1+f), a degree-2 minimax polynomial of the mantissa integer
# m = y & 0x7fffff (exact in f32). Max rel err ~6.4e-3.
EXP_SCALE = float(np.log2(np.e) * (1 << 23))
EXP_BIAS = float(127.0 * (1 << 23))
Q2 = 0.22573194345762757 / (1 << 23) ** 2
Q1 = -0.2151853848831074 / (1 << 23)
Q0 = 0.993559438904892
MANT_MASK = 0x007FFFFF

_AF = mybir.ActivationFunctionType


class _TC(tile.TileContext):
    """TileContext legalized for a walrus that allows one sync-wait per inst.

    The walrus build in this container rejects any instruction carrying more
    than one sync-wait command. After Tile's normal scheduling + the exit
    drain/barrier, rewrite every multi-wait instruction into a chain of
    same-engine NOPs (one wait each) followed by the instruction with the
    final wait.
    """

    def _drain_and_barrier(self, tick_clock, wait_clock):
        super()._drain_and_barrier(tick_clock, wait_clock)
        self._split_multi_waits()

    def _fresh_nop(self, engine):
        inst = self.nc.engines[engine].nop(nofuse=True).ins
        self.nc.cur_bb.bb.instructions.remove(inst)
        return inst

    def _split_multi_waits(self):
        for fn in self.nc.m.functions:
            for bb in fn.blocks:
                snapshot = list(bb.instructions)
                if not any(
                    inst.sync_info and len(inst.sync_info.on_wait) > 1
                    for inst in snapshot
                ):
                    continue
                new = []
                for inst in snapshot:
                    si = inst.sync_info
                    if si is not None and si.on_wait and len(si.on_wait) > 1:
                        waits = list(si.on_wait)
                        si.on_wait = waits[-1:]
                        inst.sync_info = si
                        for wt in waits[:-1]:
                            nop = self._fresh_nop(inst.engine)
                            nop.sync_info = mybir.SyncInfo(on_wait=[wt], on_update=[])
                            new.append(nop)
                    new.append(inst)
                bb.instructions[:] = new


def build_nc():
    nc = bass.Bass("TRN2")
    wf = nc.dram_tensor("wf", [WPC, A], mybir.dt.float32, kind="ExternalInput")
    tau_in = nc.dram_tensor("tau", [F], mybir.dt.float32, kind="ExternalInput")
    # W (64) and b packed into one 512-byte row so a single broadcast-gather
    # fetches both with ~100ns latency.
    wb_in = nc.dram_tensor("wb", [128], mybir.dt.float32, kind="ExternalInput")
    out = nc.dram_tensor("out", [G, 128, F], mybir.dt.bfloat16, kind="ExternalOutput")

    f32 = mybir.dt.float32
    bf16 = mybir.dt.bfloat16
    i16 = mybir.dt.int16
    i32 = mybir.dt.int32

    # Output DMA issuers: SP carries ~5/8 of the write stream, GPSIMD
    # (which also runs the tau gathers) the rest.
    _OUT_PAT = "sgssgssg"

    def out_engine(i):
        return nc.sync if _OUT_PAT[i % len(_OUT_PAT)] == "s" else nc.gpsimd

    tau_ap = tau_in[:]
    n_uses = {}
    for kind, sz, _ in SCHED:
        n_uses[sz] = n_uses.get(sz, 0) + 1

    with _TC(nc) as tc:
        with (
            tc.tile_pool(name="const", bufs=1) as const,
            tc.tile_pool(name="main", bufs=2) as main,
        ):
            def gather_rep(k, f0, sz, nsub=1):
                """Replicate tau[f0:f0+sz] to all 128 partitions via dma_gather."""
                bufs = min(2, n_uses[sz])
                rep = main.tile(
                    [128, sz], f32, tag=f"rep{sz}", name=f"rep{k}", bufs=bufs
                )
                sub = sz // nsub
                assert f0 % GATHER_STEP == 0 and sub % GATHER_STEP == 0
                for j in range(nsub):
                    idx = const.tile([128, 8], i16, tag=f"idx{k}_{j}")
                    nc.vector.memset(idx, (f0 + j * sub) // GATHER_STEP)
                    nrows = (F - sub) // GATHER_STEP + 1
                    piece = rep[:, j * sub : (j + 1) * sub]
                    out3 = bass.AP(
                        tensor=piece.tensor,
                        offset=piece.offset,
                        ap=[list(piece.ap[0]), [sub, 1], [1, sub]],
                    )
                    in2 = bass.AP(
                        tensor=tau_ap.tensor,
                        offset=tau_ap.offset,
                        ap=[[GATHER_STEP, nrows], [1, sub]],
                    )
                    nc.gpsimd.dma_gather(
                        out_ap=out3,
                        in_ap=in2,
                        idxs_ap=idx[:],
                        num_idxs=128,
                        num_idxs_reg=128,
                        elem_size=sub,
                        elem_step=GATHER_STEP,
                    )
                return rep

            # ---- warm the ACT exp/ln table before anything depends on it ----
            dummy = const.tile([128, 1], f32)
            nc.vector.memset(dummy, 0.0)
            dummy2 = const.tile([128, 1], f32)
            nc.scalar.activation(dummy2, dummy, _AF.Exp)

            # GPSIMD ucode library with DMAGather, then the input gathers:
            # wf rows p+128g -> partition p (iota indices), W|b broadcast.
            # Gather latency is ~100ns vs ~1.7us for a dma_start, so the
            # whole preamble dependency chain starts almost immediately.
            idxgs = []
            for g in range(G):
                raw = const.tile([128, 8], i16, tag=f"idxwfr{g}")
                nc.gpsimd.iota(
                    raw, pattern=[[16, 8]], base=128 * g, channel_multiplier=1
                )
                # Only idx partitions 0-15 are consumed; mask the rest into
                # range (row count 256) to satisfy bounds checks.
                idxg = const.tile([128, 8], i16, tag=f"idxwf{g}")
                nc.vector.tensor_scalar(
                    idxg, raw, scalar1=WPC - 1, scalar2=None,
                    op0=mybir.AluOpType.bitwise_and,
                )
                idxgs.append(idxg)
            nc.gpsimd.load_library(library_config.mlp)

            wf_sb = const.tile([128, G, A], f32)
            wf_ap = wf[:]
            for g in range(G):
                idxg = idxgs[g]
                piece = wf_sb[:, g, :]
                nc.gpsimd.dma_gather(
                    out_ap=bass.AP(
                        tensor=piece.tensor,
                        offset=piece.offset,
                        ap=[list(piece.ap[0]), [A, 1], [1, A]],
                    ),
                    in_ap=bass.AP(
                        tensor=wf_ap.tensor,
                        offset=wf_ap.offset,
                        ap=[[A, WPC], [1, A]],
                    ),
                    idxs_ap=idxg[:],
                    num_idxs=128,
                    num_idxs_reg=128,
                    elem_size=A,
                    elem_step=A,
                )
            wb_sb = const.tile([128, 128], f32)
            idx0 = const.tile([128, 8], i16, tag="idxwb")
            nc.vector.memset(idx0, 0)
            wb_ap = wb_in[:]
            nc.gpsimd.dma_gather(
                out_ap=bass.AP(
                    tensor=wb_sb.tensor,
                    offset=wb_sb.offset,
                    ap=[list(wb_sb.ap[0]), [128, 1], [1, 128]],
                ),
                in_ap=bass.AP(
                    tensor=wb_ap.tensor, offset=wb_ap.offset, ap=[[128, 1], [1, 128]]
                ),
                idxs_ap=idx0[:],
                num_idxs=128,
                num_idxs_reg=128,
                elem_size=128,
                elem_step=128,
            )
            w_sb = wb_sb[:, :A]
            b_sb = wb_sb[:, A : A + 1]

            rep_first = gather_rep(0, 0, SCHED[0][1])

            dve_off = {}
            f0 = 0
            for k, (kind, sz, _) in enumerate(SCHED):
                dve_off[k] = f0
                f0 += sz

            # first dve chunk's tau replication on the otherwise-idle SP
            # queue at t=0; the later dve chunks use gathers like act chunks.
            rep_d1 = main.tile(
                [128, SCHED[1][1]], f32, tag=f"rep{SCHED[1][1]}", name="rep1", bufs=2
            )
            nc.sync.dma_start(
                out=rep_d1,
                in_=bass.AP(
                    tensor=tau_ap.tensor,
                    offset=tau_ap.offset + dve_off[1],
                    ap=[[0, 128], [1, SCHED[1][1]]],
                ),
            )
            dve_reps = {1: rep_d1}

            # ---- per-worker scalars: a = x + ln15, c = -(x + b + ln(1+e^-(x+b))) - ln15
            x = const.tile([128, G], f32)
            for g in range(G):
                prod = const.tile([128, A], f32, tag=f"prod{g}")
                nc.vector.tensor_mul(prod, wf_sb[:, g, :], w_sb)
                nc.vector.reduce_sum(x[:, g : g + 1], prod, axis=mybir.AxisListType.X)

            xb = const.tile([128, G], f32)
            nc.vector.tensor_scalar(
                xb, x, scalar1=b_sb, scalar2=None, op0=mybir.AluOpType.add
            )
            e = const.tile([128, G], f32)
            nc.scalar.activation(e, xb, _AF.Exp, scale=-1.0)
            s = const.tile([128, G], f32)
            nc.scalar.activation(s, e, _AF.Identity, bias=1.0)
            ls = const.tile([128, G], f32)
            nc.scalar.activation(ls, s, _AF.Ln)
            u = const.tile([128, G], f32)
            nc.vector.tensor_add(u, xb, ls)
            c_sb = const.tile([128, G], f32)
            nc.vector.tensor_scalar(
                c_sb,
                u,
                scalar1=-1.0,
                scalar2=-LN15,
                op0=mybir.AluOpType.mult,
                op1=mybir.AluOpType.add,
            )
            a_sb = const.tile([128, G], f32)
            nc.vector.tensor_scalar_add(a_sb, xb, LN15)

            # scaled affine constants for the DVE bitcast-exp2 path
            a_sc = const.tile([128, G], f32)
            nc.vector.tensor_scalar_mul(a_sc, a_sb, EXP_SCALE)
            ccorr = const.tile([128, G], f32)
            nc.vector.tensor_scalar(
                ccorr,
                c_sb,
                scalar1=EXP_SCALE,
                scalar2=EXP_BIAS,
                op0=mybir.AluOpType.mult,
                op1=mybir.AluOpType.add,
            )

            def dve_exp(ot, rep, d, g, key):
                # bf16 poly intermediates hit the DVE 2x (2-byte packed)
                # mode; precision stays within budget (max ~1.1e-2 overall).
                y = main.tile([128, d], i32, tag=f"dve_y{d}", name=f"y{key}", bufs=1)
                nc.vector.tensor_scalar(
                    y,
                    rep,
                    scalar1=a_sc[:, g : g + 1],
                    scalar2=ccorr[:, g : g + 1],
                    op0=mybir.AluOpType.mult,
                    op1=mybir.AluOpType.add,
                )
                m = main.tile([128, d], i32, tag=f"dve_m{d}", name=f"m{key}", bufs=1)
                nc.vector.tensor_scalar(
                    m, y, scalar1=MANT_MASK, scalar2=None, op0=mybir.AluOpType.bitwise_and
                )
                fm = main.tile([128, d], bf16, tag=f"dve_fm{d}", name=f"fm{key}", bufs=1)
                nc.vector.tensor_scalar_add(fm, m, 0.0)
                h1 = main.tile([128, d], bf16, tag=f"dve_h1{d}", name=f"h1{key}", bufs=1)
                nc.vector.tensor_scalar(
                    h1,
                    fm,
                    scalar1=Q2,
                    scalar2=Q1,
                    op0=mybir.AluOpType.mult,
                    op1=mybir.AluOpType.add,
                )
                h2 = main.tile([128, d], bf16, tag=f"dve_h2{d}", name=f"h2{key}", bufs=1)
                nc.vector.tensor_mul(h2, h1, fm)
                h3 = main.tile([128, d], bf16, tag=f"dve_h3{d}", name=f"h3{key}", bufs=1)
                nc.vector.tensor_scalar_add(h3, h2, Q0)
                nc.vector.tensor_mul(ot, h3, y.bitcast(f32))

            # ---- main loop ----
            out_i = 0
            f0 = 0
            for k, (kind, sz, nout) in enumerate(SCHED):
                if k == 0:
                    rep = rep_first
                elif k in dve_reps:
                    rep = dve_reps[k]
                else:
                    rep = gather_rep(k, f0, sz, nsub=2 if k in ACT_SPLIT else 1)
                bufs = min(2, n_uses[sz])
                for g in range(G):
                    ot = main.tile(
                        [128, sz],
                        bf16,
                        tag=f"ot{sz}g{g}",
                        name=f"ot{k}g{g}",
                        bufs=bufs,
                    )
                    osub = sz // nout

                    def emit_outs(lo, hi, g=g, ot=ot, base=f0):
                        nonlocal out_i
                        for j in range(lo, hi):
                            out_engine(out_i).dma_start(
                                out=out[g, :, base + j * osub : base + (j + 1) * osub],
                                in_=ot[:, j * osub : (j + 1) * osub],
                            )
                            out_i += 1

                    if kind == "act":
                        nsp = 2 if k in ACT_SPLIT else 1
                        half = sz // nsp
                        for h in range(nsp):
                            nc.scalar.activation(
                                ot[:, h * half : (h + 1) * half],
                                rep[:, h * half : (h + 1) * half],
                                _AF.Exp,
                                bias=c_sb[:, g : g + 1],
                                scale=a_sb[:, g : g + 1],
                            )
                            emit_outs(
                                h * nout // nsp, (h + 1) * nout // nsp
                            )
                    else:
                        dve_exp(ot, rep, sz, g, f"{k}g{g}")
                        emit_outs(0, nout)
                f0 += sz
    return nc


_NC = None


def kernel(inputs, W, b, worker_num=WN, task_num=TN, edge_type=L, ability_num=A, **_kw):
    global _NC
    inputs = np.ascontiguousarray(np.asarray(inputs, dtype=np.float32))
    W = np.asarray(W, dtype=np.float32).reshape(A)
    b = np.asarray(b, dtype=np.float32).reshape(1)
    assert inputs.shape == (WN + TN, A)

    wf = inputs[:WN, :A]
    tau = np.ascontiguousarray(inputs[WN:, :L].reshape(F))
    wb = np.zeros(128, dtype=np.float32)
    wb[:A] = W
    wb[A] = b[0]

    if _NC is None:
        _NC = build_nc()

    in_maps = [
        {
            "wf": np.ascontiguousarray(wf[k * WPC : (k + 1) * WPC]),
            "tau": tau,
            "wb": wb,
        }
        for k in range(NCORES)
    ]
    res = run_bass_kernel_spmd(_NC, in_maps, core_ids=list(range(NCORES)))
    parts = [
        np.asarray(r["out"]).astype(np.float32).reshape(WPC, TN, L)
        for r in res.results
    ]
    return np.concatenate(parts, axis=0)
